# revision 1
# baseline (speedup 1.0000x reference)
"""Causal attention head (B=16, S=2048, d=64) on 8 TRN2 NeuronCores.

Data parallel over batch: each core gets 2 batches and computes its full
S x S causal attention.

Per-core algorithm (transposed-scores layout), v2:
  scores_T[j, i] = sum_d k[j,d] q[i,d]            (j on PSUM partitions, bf16)
  attn_T = exp(scores_T / 64)  (scores tiny: no max-subtraction needed)
  out[i, :64], l[i] = sum_j attn_T[j, i] * [v[j, :] | 1]   (ones-column)
  out[i] /= l[i]

v2 changes vs v1:
  - scores PSUM tiles are bf16 [P, 2, 2, 512] (2 j-chunk pairs = 2 banks),
    exp covers 2 pairs per ACT instruction (halves ACT instruction count)
  - diagonal masking via a [128,128] triangle-constant multiply on the
    [128,128] diagonal blocks only (was: full-tile gpsimd affine_select)
  - the all-masked head regions of diagonal chunks are skipped in mm2
    (narrowed rhs) instead of being zeroed
  - k/q transposes write bf16 PSUM so the PSUM->SBUF copies run in the
    DVE 2x perf mode; q dup copy is SBUF->SBUF at 4x
  - v is loaded via SWDGE casting DMA directly into the bf16 vp tile
  - q/k loads are 2 big DMAs per tensor instead of 4
"""

import numpy as np

import concourse.bacc as bacc
import concourse.bass as bass
import concourse.mybir as mybir
import concourse.tile as tile
from concourse.bass_utils import run_bass_kernel_spmd
from concourse.masks import make_identity

F32 = mybir.dt.float32
BF16 = mybir.dt.bfloat16

B, S, D = 16, 2048, 64
N_CORES = 8
BPC = B // N_CORES  # batches per core
P = 128
ITILE = 512               # i-tile width (free dim of scores_T)
N_IT = S // ITILE         # 4 i-tiles
N_JC = S // P             # 16 j-chunks
SCALE = 1.0 / D
QA = (0.240723 ** 0.5) / D   # sqrt(c)/64
QB = (0.240723 ** 0.5) * 2.133  # sqrt(c)*t

import os as _os

QUAD_BUFS = int(_os.environ.get("K_QUAD_BUFS", "2"))
ACC_BUFS = int(_os.environ.get("K_ACC_BUFS", "2"))
TRP_BUFS = int(_os.environ.get("K_TRP_BUFS", "2"))
LAG_N = int(_os.environ.get("K_LAG", "3"))
ATTN_BUFS = int(_os.environ.get("K_ATTN_BUFS", "4"))
VCAST = int(_os.environ.get("K_VCAST", "1"))  # v via SWDGE casting DMA
MASK_DVE = int(_os.environ.get("K_MASK_DVE", "1"))  # tri-mask on DVE not Pool
# exp offload: this many of the 40 score tiles use a quadratic approx
# exp(s) ~= c*(s+t)^2 computed DVE(tensor_scalar) + Pool(square) instead of
# the ACT exp LUT.  Fit on [-0.9, 0.9], max rel err ~10%, cancels to <1%
# after softmax normalization (validated end-to-end in numpy vs reference).
QOFF_N = int(_os.environ.get("K_QOFF", "0"))
FDRAIN = int(_os.environ.get("K_FDRAIN", "1"))  # incremental final-tile drain
UNROLL = int(_os.environ.get("K_UNROLL", "8"))
QC = 0.240723  # c
QT = 2.133     # t

# pairs per batch
GPB = N_IT * (N_IT + 1)  # 20


def build_kernel(loop: int = 0):
    nc = bacc.Bacc("TRN2", target_bir_lowering=False, debug=False)
    q_h = nc.dram_tensor("q", [BPC, S, D], F32, kind="ExternalInput").ap()
    k_h = nc.dram_tensor("k", [BPC, S, D], F32, kind="ExternalInput").ap()
    v_h = nc.dram_tensor("v", [BPC, S, D], F32, kind="ExternalInput").ap()
    o_h = nc.dram_tensor("o", [BPC, S, D], F32, kind="ExternalOutput").ap()

    with tile.TileContext(nc) as tc:
        with (
            tc.tile_pool(name="const", bufs=1) as const,
            tc.tile_pool(name="stage", bufs=2) as stage,
            tc.tile_pool(name="qkt", bufs=2) as qkt,
            tc.tile_pool(name="attn", bufs=ATTN_BUFS) as attnp,
            tc.tile_pool(name="usq", bufs=2) as usqp,
            tc.tile_pool(name="outs", bufs=4) as outs,
            tc.tile_pool(name="quad", bufs=QUAD_BUFS, space="PSUM") as quadp,
            tc.tile_pool(name="acc", bufs=ACC_BUFS, space="PSUM") as accp,
            tc.tile_pool(name="trp", bufs=TRP_BUFS, space="PSUM") as trp,
        ):
            ident_f = const.tile([P, P], F32)
            make_identity(nc, ident_f)
            ident_b = const.tile([P, P], BF16)
            nc.vector.tensor_copy(ident_b, ident_f)
            # triangle constant: tri[j, x] = 1 if x >= j else 0
            tri = const.tile([P, P], BF16)
            nc.gpsimd.memset(tri, 1.0)
            nc.gpsimd.affine_select(
                out=tri,
                in_=tri,
                compare_op=mybir.AluOpType.is_ge,
                fill=0.0,
                base=0,
                pattern=[[1, P]],
                channel_multiplier=-1,
            )
            # warm the ACT exp table while the input DMAs run
            warm = const.tile([P, 1], F32)
            nc.scalar.activation(
                warm, ident_f[:, 0:1], mybir.ActivationFunctionType.Exp
            )

            mask_eng = nc.vector if MASK_DVE else nc.gpsimd

            def stage_a_loads(b):
                # q/k: fp32 HWDGE loads, then one DVE cast per tensor to a
                # bf16 staging tile (transposes and mm1 only need bf16)
                qf = stage.tile([P, N_JC, D], F32, tag="qf", name=f"qf{b}")
                kf = stage.tile([P, N_JC, D], F32, tag="kf", name=f"kf{b}")
                qn = stage.tile([P, N_JC, D], BF16, tag="qn", name=f"qn{b}")
                kn = stage.tile([P, N_JC, D], BF16, tag="kn", name=f"kn{b}")
                vp = stage.tile([P, N_JC, D + 1], BF16, tag="vp", name=f"vp{b}")
                kr = k_h[b].rearrange("(n p) d -> p n d", p=P)
                qr = q_h[b].rearrange("(n p) d -> p n d", p=P)
                vr = v_h[b].rearrange("(n p) d -> p n d", p=P)
                # first two quarters small (startup latency), then a half;
                # bf16 casts happen lazily in the transpose thunks.
                # q rides the ACT HWDGE queue, k the SP queue (parallel DGE).
                for i2, sl in enumerate(
                    (slice(0, 4), slice(4, 8), slice(8, N_JC))
                ):
                    nc.sync.dma_start(qf[:, sl, :], qr[:, sl, :])
                    keng = nc.scalar if (i2 == 0 and b == 0) else nc.sync
                    keng.dma_start(kf[:, sl, :], kr[:, sl, :])
                if VCAST:
                    nc.gpsimd.dma_start(vp[:, :, 0:D], vr)
                else:
                    vn = stage.tile([P, N_JC, D], F32, tag="vn", name=f"vn{b}")
                    for h in range(2):
                        sl = slice(8 * h, 8 * (h + 1))
                        nc.sync.dma_start(vn[:, sl, :], vr[:, sl, :])
                        nc.gpsimd.tensor_copy(vp[:, sl, 0:D], vn[:, sl, :])
                nc.gpsimd.memset(vp[:, :, D : D + 1], 1.0)
                return qf, kf, qn, kn, vp

            def one_pass(warm=True):
                staged = [stage_a_loads(b) for b in range(BPC)]
                if warm:
                    # keep PE busy while the first loads land (P-state warmup)
                    wtr = trp.tile([P, P], BF16, tag="trp", name="warmtr")
                    for _ in range(12):
                        nc.tensor.transpose(wtr, ident_b, ident_b)
                kt2s, qts = [], []
                thunks = []  # per batch: dict name -> thunk
                for b in range(BPC):
                    kt2s.append(
                        qkt.tile([P, S // 2], BF16, tag="kt", name=f"kt{b}")
                    )
                    qts.append(qkt.tile([P, S], BF16, tag="qt", name=f"qt{b}"))

                def k_pair(b, p):
                    qf, kf, qn, kn, vp = staged[b]
                    qsl = slice(2 * p, 2 * (p + 1))
                    nc.vector.tensor_copy(kn[:, qsl, :], kf[:, qsl, :])
                    tr = trp.tile(
                        [P, P], BF16, tag="trp", name=f"trk{b}_{p}"
                    )
                    nc.tensor.transpose(
                        tr, kn[:, 2 * p : 2 * p + 2, :], ident_b
                    )
                    nc.vector.tensor_copy(
                        kt2s[b][:, P * p : P * (p + 1)], tr
                    )

                def q_group(b, it):
                    qf, kf, qn, kn, vp = staged[b]
                    qsl = slice(4 * it, 4 * (it + 1))
                    nc.vector.tensor_copy(qn[:, qsl, :], qf[:, qsl, :])
                    tr = trp.tile(
                        [P, 4, P], BF16, tag="trp", name=f"trq{b}_{it}"
                    )
                    for u in range(4):
                        nc.tensor.transpose(
                            tr[0:D, u, :], qn[:, 4 * it + u, :], ident_b
                        )
                    sl = slice(ITILE * it, ITILE * (it + 1))
                    nc.vector.tensor_copy(qts[b][0:D, sl], tr[0:D])
                    nc.vector.tensor_copy(qts[b][D : 2 * D, sl], qts[b][0:D, sl])

                # batch 0 walks i-tiles ascending (cheap warmup), the last
                # batch descending (its final i-tile has only 2 pairs ->
                # short drain)
                def it_order(b):
                    return (
                        range(N_IT)
                        if b < BPC - 1
                        else range(N_IT - 1, -1, -1)
                    )

                # thunk schedule: per batch-local pair index -> thunks to emit
                sched = {b: {} for b in range(BPC)}
                INJ0 = int(_os.environ.get("K_INJ", "16"))
                AHEAD = int(_os.environ.get("K_AHEAD", "2"))
                thunk_order = {}
                for b in range(BPC):
                    seq = [
                        (it, pr)
                        for it in it_order(b)
                        for pr in range(2 * (it + 1))
                    ]
                    seen = []
                    for j, (it, pr) in enumerate(seq):
                        for nm in (f"q{it}", f"k{pr}"):
                            if nm not in seen:
                                seen.append(nm)
                                js = j if j == 0 else max(1, j - AHEAD)
                                sched[b].setdefault(js, []).append((b, nm))
                    thunk_order[b] = seen
                for b in range(BPC - 1):
                    for i, nm in enumerate(thunk_order[b + 1]):
                        sched[b].setdefault(INJ0 + i, []).append((b + 1, nm))
                emitted = set()

                def emit_thunk(key):
                    if key in emitted:
                        return
                    emitted.add(key)
                    tb, nm = key
                    if nm[0] == "k":
                        k_pair(tb, int(nm[1:]))
                    else:
                        q_group(tb, int(nm[1:]))

                # pair-granular pipeline: pair pr covers j-chunks 2pr,2pr+1
                groups = [
                    (b, it, pr)
                    for b in range(BPC)
                    for it in it_order(b)
                    for pr in range(2 * (it + 1))
                ]
                LAG = LAG_N
                qoff = (
                    {round(i * GPB * BPC / QOFF_N) for i in range(QOFF_N)}
                    if QOFF_N
                    else set()
                )
                acc_by = {}
                atq = {}
                pend_c2 = []

                def stage_c_part(b, it, osb, s0, ns, eng=None):
                    trq = trp.tile(
                        [P, ns, D + 1], F32, tag="trp",
                        name=f"tro{b}_{it}_{s0}",
                    )
                    for s4 in range(ns):
                        nc.tensor.transpose(
                            trq[:, s4, :],
                            osb[:, P * s4 : P * (s4 + 1)],
                            ident_f[0 : D + 1, 0 : D + 1],
                        )
                    rec = outs.tile([P, ns], F32, tag="rec")
                    nc.vector.reciprocal(rec, trq[:, :, D])
                    fin = outs.tile([P, ns, D], F32, tag="fin")
                    nc.vector.tensor_tensor(
                        fin,
                        trq[:, :, 0:D],
                        rec[:, :, None].to_broadcast((P, ns, D)),
                        mybir.AluOpType.mult,
                    )
                    r0 = ITILE * it + P * s0
                    (eng or nc.sync).dma_start(
                        o_h[b, r0 : r0 + P * ns, :].rearrange(
                            "(s p) d -> p s d", p=P
                        ),
                        fin,
                    )

                def stage_c2(last=False):
                    while pend_c2:
                        b, it, osb = pend_c2.pop(0)
                        stage_c_part(
                            b, it, osb, 0, 4,
                            eng=nc.scalar if last else None,
                        )

                HALF = ITILE // 2

                def emit_mm2(idx):
                    b, it, pr = groups[idx]
                    at = atq.pop(idx)
                    vp = staged[b][4]
                    acc = acc_by[(b, it)]
                    diag = pr >= 2 * it
                    u = pr - 2 * it  # 0/1 within the diagonal pair block
                    if isinstance(acc, tuple):
                        # final i-tile: acc split into two half-width banks
                        # so the low half drains while the last pair runs
                        acc_a, acc_b = acc
                        for c in range(2):
                            jc = 2 * pr + c
                            lo = 128 * (2 * u + c) if diag else 0
                            if lo < HALF:
                                nc.tensor.matmul(
                                    acc_a[:, lo:],
                                    lhsT=vp[:, jc, :],
                                    rhs=at[:, c, lo:HALF],
                                    start=(pr == 0 and c == 0),
                                    stop=(pr == 0 and c == 1),
                                )
                            nc.tensor.matmul(
                                acc_b[:, max(lo - HALF, 0) :],
                                lhsT=vp[:, jc, :],
                                rhs=at[:, c, max(lo, HALF) :],
                                start=(pr == 0 and c == 0),
                                stop=(pr == 2 * it + 1 and c == 1),
                            )
                        if pr == 0:
                            # low half complete: drain it now (ACT DGE queue)
                            osb = outs.tile([D + 1, HALF], F32, tag="osbh")
                            nc.vector.tensor_copy(osb, acc_a)
                            stage_c_part(b, it, osb, 0, 2, eng=nc.scalar)
                        else:
                            acc_by.pop((b, it))
                            osb = outs.tile([D + 1, HALF], F32, tag="osbh")
                            nc.vector.tensor_copy(osb, acc_b)
                            stage_c_part(b, it, osb, 2, 2, eng=nc.sync)
                        return
                    for c in range(2):
                        jc = 2 * pr + c
                        lo = 128 * (2 * u + c) if diag else 0
                        nc.tensor.matmul(
                            acc[:, lo:] if lo else acc,
                            lhsT=vp[:, jc, :],
                            rhs=at[:, c, lo:] if lo else at[:, c, :],
                            start=(pr == 0 and c == 0),
                            stop=(pr == 2 * it + 1 and c == 1),
                        )
                    if pr == 2 * it + 1:
                        acc_by.pop((b, it))
                        osb = outs.tile([D + 1, ITILE], F32, tag="osb")
                        nc.vector.tensor_copy(osb, acc)
                        pend_c2.append((b, it, osb))

                for idx, (b, it, pr) in enumerate(groups):
                    jloc = idx - GPB * b
                    for key in sched[b].get(jloc, []):
                        emit_thunk(key)
                    if pr == 0:
                        if FDRAIN and idx == len(groups) - 2:
                            acc_by[(b, it)] = (
                                accp.tile(
                                    [D + 1, ITILE // 2], F32, tag="acc",
                                    name=f"accA{b}_{it}",
                                ),
                                accp.tile(
                                    [D + 1, ITILE // 2], F32, tag="acc",
                                    name=f"accB{b}_{it}",
                                ),
                            )
                        else:
                            acc_by[(b, it)] = accp.tile(
                                [D + 1, ITILE], F32, tag="acc",
                                name=f"acc{b}_{it}",
                            )
                    st = quadp.tile([P, 2, ITILE], F32, tag="quad")
                    for c in range(2):
                        h = slice(D * c, D * (c + 1))
                        nc.tensor.matmul(
                            st[:, c, :],
                            lhsT=kt2s[b][h, P * pr : P * (pr + 1)],
                            rhs=qts[b][h, ITILE * it : ITILE * (it + 1)],
                            start=True,
                            stop=True,
                        )
                    at = attnp.tile([P, 2, ITILE], BF16, tag="attn")
                    if pr == 2 * it + 1:
                        # odd diagonal pair: i-cols < 256 of chunk c=0 are
                        # fully above the diagonal (mm2 skips them), so exp
                        # only the flat range [256, 1024)
                        stf = st.rearrange("p c i -> p (c i)")
                        atf = at.rearrange("p c i -> p (c i)")
                        nc.scalar.activation(
                            atf[:, 2 * P : 2 * ITILE],
                            stf[:, 2 * P : 2 * ITILE],
                            mybir.ActivationFunctionType.Exp,
                            scale=SCALE,
                        )
                    elif idx in qoff:
                        # quadratic approx: at = (s*a + b)^2 on DVE + Pool
                        usq = usqp.tile([P, 2, ITILE], BF16, tag="usq")
                        nc.vector.tensor_scalar(
                            usq, st, QA, QB,
                            mybir.AluOpType.mult, mybir.AluOpType.add,
                        )
                        nc.gpsimd.tensor_tensor(
                            at, usq, usq, mybir.AluOpType.mult
                        )
                    else:
                        nc.scalar.activation(
                            at, st, mybir.ActivationFunctionType.Exp,
                            scale=SCALE,
                        )
                    if pr >= 2 * it:
                        # diagonal pair: mask the two [128,128] triangle
                        # blocks (chunk cc's block sits at i-cols 128*cc).
                        u = pr - 2 * it
                        for c in range(2):
                            cc = 2 * u + c
                            sl = slice(P * cc, P * (cc + 1))
                            mask_eng.tensor_tensor(
                                at[:, c, sl],
                                at[:, c, sl],
                                tri,
                                mybir.AluOpType.mult,
                            )
                    atq[idx] = at
                    stage_c2()
                    if idx == len(groups) - 1:
                        # drain: no lag on the final groups
                        for j2 in range(idx - LAG, idx + 1):
                            emit_mm2(j2)
                            stage_c2(last=(j2 == idx))
                    elif idx >= LAG:
                        emit_mm2(idx - LAG)
                stage_c2()

            if loop > 0:
                hints = (
                    mybir.EngineType.PE,
                    mybir.EngineType.Activation,
                    mybir.EngineType.DVE,
                    mybir.EngineType.Pool,
                    mybir.EngineType.SP,
                )
                if _os.environ.get("K_LOOP_HINTS", "1") == "0":
                    hints = ()
                # UNROLL passes per loop iteration: consecutive passes
                # pipeline through the tile pools (no all-engine barrier
                # between them), so startup/drain amortize.
                assert loop % UNROLL == 0, (loop, UNROLL)
                stag = bool(int(_os.environ.get("K_STAGGER", "1")))
                with tc.For_i(
                    0, loop // UNROLL, 1, hint_engines=hints,
                    staggered_reset=stag,
                ):
                    # no PE warmup inside the steady-state loop: the 12
                    # dummy transposes are too short to flip HAM and just
                    # add ~0.65us/iter
                    for i in range(UNROLL):
                        one_pass(warm=False)
            elif loop < 0:
                # sim-only: -N emits N back-to-back passes without a loop
                for i in range(-loop):
                    one_pass(warm=(i == 0))
            else:
                one_pass()

    nc.compile()
    return nc


_CACHE: dict = {}


def _get_nc(loop: int = 0):
    if loop not in _CACHE:
        _CACHE[loop] = build_kernel(loop)
    return _CACHE[loop]


def kernel(q: np.ndarray, k: np.ndarray, v: np.ndarray) -> np.ndarray:
    q = np.ascontiguousarray(q, dtype=np.float32)
    k = np.ascontiguousarray(k, dtype=np.float32)
    v = np.ascontiguousarray(v, dtype=np.float32)
    nc = _get_nc(0)
    in_maps = [
        {
            "q": q[BPC * i : BPC * (i + 1)],
            "k": k[BPC * i : BPC * (i + 1)],
            "v": v[BPC * i : BPC * (i + 1)],
        }
        for i in range(N_CORES)
    ]
    res = run_bass_kernel_spmd(nc, in_maps, list(range(N_CORES)))
    return np.concatenate([res.results[i]["o"] for i in range(N_CORES)], axis=0)

